# revision 24
# baseline (speedup 1.0000x reference)
"""MessagePassingConvolution kernel for 8 Trainium2 NeuronCores (v2).

Strategy (all-bf16, deep software pipeline, engine-balanced):
  - Host: nodes are LPT bin-packed into 80 balanced windows of <=128 nodes
    and <=2048 edges (16 subtiles of 128; data max ~2010), so the per-core
    edge budget drops to 20480 (vs 21760 with naive contiguous windows)
    and every window is exactly 4 tiles of 512 edges - no ragged tiles,
    no window-straddling tiles. Core m owns windows 10m..10m+9; the host
    inverse-permutes the output rows at the end.
  - The per-edge einsum u[lo,e] = sum_ki Wg[ki,lo] h3[k,e] x[i,e] uses the
    ki -> (group, partition) split k = 16g + p//8, i = 8cx + p%8 (IB=8):
    4 x-replication tables streamed from host (xs, 2 bytes * 512 rows/edge
    = half the DMA of the IB=4 split) and 4 h-replication patterns that
    fold into the third MLP layer: zb_g = (W3 R_g).T @ h2 on the PE, then
    one Act silu per group evacuates psum->SBUF bf16 (silu commutes with
    the 0/1 replication R). The DVE Hadamard A = hbs * xs runs in 2x
    16-bit all-SBUF mode as TWO ops per tile (groups 0-2 merged into one
    [128,3,4,512] op); group 3's last 2 chunks run on the otherwise-idle
    GpSimd engine. The PE then accumulates u[96,512] += Wg_c.T @ A_c over
    the 16 chunks.
  - Output side: GpSimd evacuates u to bf16, PE transposes to edge-major,
    DVE multiplies by host-pre-expanded edge_attrs (atx, c-replicated so
    the multiply runs in 2x mode with unit-stride operands), and the
    scatter accumulates psum_acc[128,288] += S_st.T @ msgs_st over a
    window's 16 subtiles. Scatter masks are host-precomputed and DMA'd
    (beats on-chip is_equal, which runs at 1x due to a stride-0 operand).
  - Every pipeline stage has a full iteration of slack: iteration gt runs
    mlp(gt+3), zb+silu(gt+2), Hadamard(gt+1), einsum(gt), transpose/msgs
    (gt-1), scatter(gt-2). Bulk xs DMA rides the GpSimd queue (25ns
    dispatch vs 565ns on sync); ef/atx/masks/out ride sync/scalar.
  - Output: per-core [1280, 288] slices -> concat -> inverse node
    permutation -> [10000, 32, 9].
"""

import sys
import numpy as np
import heapq
from contextlib import ExitStack

sys.path.insert(0, "/opt/trn_rl_repo")

import concourse.bass as bass  # noqa: E402
import concourse.bacc as bacc  # noqa: E402
import concourse.mybir as mybir  # noqa: E402
import concourse.tile as tile  # noqa: E402
from concourse.bass_utils import run_bass_kernel_spmd  # noqa: E402

import ml_dtypes  # noqa: E402

BF16 = ml_dtypes.bfloat16

# ---- problem constants (hardcoded per spec) ----
N_NODES = 10000
N_EDGES = 160000
C = 32
RADIAL = 8
HID = 64
NL = 3
L_DIMS = (1, 3, 5)
NSH = 9
AVG_NUM_NEIGHBORS = 16.0

N_CORES = 8
WIN = 128                      # nodes per window (psum partitions)
WINS_PER_CORE = 10
N_WINS = N_CORES * WINS_PER_CORE            # 80
SUB = 128                      # edges per subtile
SUBS_PER_WIN = 16              # window edge budget = 2048 (balanced packing)
WIN_E = SUB * SUBS_PER_WIN     # 2048
E_CORE = WIN_E * WINS_PER_CORE              # 20480
N_SUBTILES = WINS_PER_CORE * SUBS_PER_WIN   # 160
TSZ = 512                      # edges per tile (4 subtiles)
N_TILES = N_SUBTILES // 4                   # 40 (window = exactly 4 tiles)

IB = 4                         # i-values per chunk partition period
KA = 128 // IB                 # k-rows per group = 32
NXT = C // IB                  # 8 x-replication tables
NGRP = HID // KA               # 2 h-groups
N_CHUNK = NGRP * NXT           # 16 ki-chunks
GP_CX = 0                      # chunks of the last group computed on GpSimd
                               # (0: measured GpSimd offload inflates the
                               # concurrent DVE op by more than it saves)
LO = NL * C                    # 96
F_OUT = NSH * C                # 288

FP32 = mybir.dt.float32
BF16_DT = mybir.dt.bfloat16

_CACHED = {}

ACT_FUNC = mybir.ActivationFunctionType.Silu


def _build_nc():
    nc = bacc.Bacc()

    ef = nc.dram_tensor("ef", [RADIAL, E_CORE], BF16_DT, kind="ExternalInput")
    xs = nc.dram_tensor("xs", [128, NXT * E_CORE], BF16_DT, kind="ExternalInput")
    atx = nc.dram_tensor("atx", [SUB, N_SUBTILES * NSH * C], BF16_DT,
                         kind="ExternalInput")
    mk = nc.dram_tensor("mk", [SUB, WINS_PER_CORE * SUBS_PER_WIN * WIN], BF16_DT,
                        kind="ExternalInput")
    w1 = nc.dram_tensor("w1", [RADIAL, HID], BF16_DT, kind="ExternalInput")
    w2 = nc.dram_tensor("w2", [HID, HID], BF16_DT, kind="ExternalInput")
    w3r = nc.dram_tensor("w3r", [HID, NGRP * 128], BF16_DT, kind="ExternalInput")
    wg = nc.dram_tensor("wg", [128, N_CHUNK * LO], BF16_DT, kind="ExternalInput")
    rm = nc.dram_tensor("rm", [LO, F_OUT], BF16_DT, kind="ExternalInput")
    out = nc.dram_tensor("out", [WIN * WINS_PER_CORE, F_OUT], FP32,
                         kind="ExternalOutput")

    with tile.TileContext(nc) as tc, ExitStack() as ctx:
        const_p = ctx.enter_context(tc.tile_pool(name="const", bufs=1))
        stream5 = ctx.enter_context(tc.tile_pool(name="s5", bufs=5))
        stream3 = ctx.enter_context(tc.tile_pool(name="s3", bufs=3))
        stream2 = ctx.enter_context(tc.tile_pool(name="s2", bufs=3))
        win_p = ctx.enter_context(tc.tile_pool(name="win", bufs=4))
        psum_mlp = ctx.enter_context(tc.tile_pool(name="pmlp", bufs=1, space="PSUM"))
        psum_zb = ctx.enter_context(tc.tile_pool(name="pzb", bufs=2, space="PSUM"))
        psum_u = ctx.enter_context(tc.tile_pool(name="pu", bufs=2, space="PSUM"))
        psum_ut = ctx.enter_context(tc.tile_pool(name="put", bufs=1, space="PSUM"))
        psum_acc = ctx.enter_context(tc.tile_pool(name="pacc", bufs=1, space="PSUM"))

        # ---- one-time constants ----
        w1_sb = const_p.tile([RADIAL, HID], BF16_DT)
        nc.sync.dma_start(w1_sb[:], w1[:])
        w2_sb = const_p.tile([HID, HID], BF16_DT)
        nc.sync.dma_start(w2_sb[:], w2[:])
        rm_sb = const_p.tile([LO, F_OUT], BF16_DT)
        nc.sync.dma_start(rm_sb[:], rm[:])
        w3r_sb = const_p.tile([HID, NGRP * 128], BF16_DT)
        nc.scalar.dma_start(w3r_sb[:], w3r[:])
        wg_sb = const_p.tile([128, N_CHUNK * LO], BF16_DT)
        nc.scalar.dma_start(wg_sb[:], wg[:])

        tstate = {}   # gt -> dict(ef, x, atx, h1, h2, hbs, a, u_sb, ut, msgs)
        wstate = {}   # w -> dict(mk, acc)
        lofs = (0, 1, 4)

        def prefetch(gt):
            """ef/atx on sync queue, bulk xs on the cheap gpsimd queue."""
            base = gt * TSZ
            ef_sb = stream5.tile([RADIAL, TSZ], BF16_DT, tag="ef", name=f"ef_{gt}")
            nc.sync.dma_start(ef_sb[:], ef[:, base:base + TSZ])
            x_sb = stream5.tile([128, NXT, TSZ], BF16_DT, tag="x", name=f"x_{gt}")
            nc.gpsimd.dma_start(
                x_sb[:].rearrange("p c e -> p (c e)"),
                xs[:, NXT * base:NXT * (base + TSZ)])
            atx_sb = stream5.tile([SUB, 4, NSH, C], BF16_DT, tag="atx",
                                  name=f"atx_{gt}")
            nc.sync.dma_start(
                atx_sb[:].rearrange("p s m c -> p (s m c)"),
                atx[:, gt * 4 * NSH * C:(gt + 1) * 4 * NSH * C])
            tstate[gt] = dict(ef=ef_sb, x=x_sb, atx=atx_sb)

        def load_masks(w):
            mk_sb = win_p.tile([SUB, SUBS_PER_WIN, WIN], BF16_DT, tag="mk",
                               name=f"mk_w{w}")
            nc.scalar.dma_start(
                mk_sb[:].rearrange("p s n -> p (s n)"),
                mk[:, w * SUBS_PER_WIN * WIN:(w + 1) * SUBS_PER_WIN * WIN])
            wstate[w] = dict(mk=mk_sb, acc=None)

        def do_mlp1(gt):
            """Layer 1 of the radial MLP for tile gt (PE + Act silu)."""
            st = tstate[gt]
            z1 = psum_mlp.tile([HID, TSZ], FP32, tag="z", name=f"z1_{gt}")
            nc.tensor.matmul(out=z1[:], lhsT=w1_sb[:], rhs=st["ef"][:],
                             start=True, stop=True, skip_group_check=True)
            h1 = stream3.tile([HID, TSZ], BF16_DT, tag="h1", name=f"h1_{gt}")
            nc.scalar.activation(h1[:], z1[:], ACT_FUNC)
            st["h1"] = h1

        def do_mlp2(gt):
            st = tstate[gt]
            z2 = psum_mlp.tile([HID, TSZ], FP32, tag="z", name=f"z2_{gt}")
            nc.tensor.matmul(out=z2[:], lhsT=w2_sb[:], rhs=st["h1"][:],
                             start=True, stop=True, skip_group_check=True)
            h2 = stream3.tile([HID, TSZ], BF16_DT, tag="h2", name=f"h2_{gt}")
            nc.scalar.activation(h2[:], z2[:], ACT_FUNC)
            st["h2"] = h2

        def do_mlp12(gt):
            do_mlp1(gt)
            do_mlp2(gt)

        def do_zb(gt, g0, g1):
            """MLP layer 3 fused with h-replication: zb_g = (W3 R_g).T @ h2,
            Act silu evacuates each group to SBUF bf16 (silu commutes with
            the 0/1 replication)."""
            st = tstate[gt]
            if g0 == 0:
                st["hbs"] = stream3.tile([128, NGRP, TSZ], BF16_DT, tag="hbs",
                                         name=f"hbs_{gt}")
            hbs = st["hbs"]
            for g in range(g0, g1):
                zb = psum_zb.tile([128, TSZ], FP32, tag="zb", name=f"zb_{gt}_{g}")
                nc.tensor.matmul(out=zb[:],
                                 lhsT=w3r_sb[:, g * 128:(g + 1) * 128],
                                 rhs=st["h2"][:],
                                 start=True, stop=True, skip_group_check=True)
                nc.scalar.activation(hbs[:, g, :], zb[:], ACT_FUNC)

        def do_had_gp(gt):
            """GpSimd's share of the Hadamard (last GP_CX chunks of the last
            group), staggered one iteration ahead of the DVE share so the
            two engines never contend for the same SBUF tiles. With
            GP_CX=0 this just allocates the A tile."""
            st = tstate[gt]
            a = stream2.tile([128, NGRP, NXT, TSZ], BF16_DT, tag="a",
                             name=f"a_{gt}")
            if GP_CX:
                dg = NGRP - 1
                nv = NXT - GP_CX
                nc.gpsimd.tensor_tensor(
                    out=a[:, dg, nv:, :],
                    in0=st["hbs"][:, dg, None, :].to_broadcast(
                        [128, GP_CX, TSZ]),
                    in1=st["x"][:, nv:, :],
                    op=mybir.AluOpType.mult)
            st["a"] = a

        def do_had_dve(gt):
            """DVE share of the Hadamard: one 3D 2x-mode op per group (the
            exact AP shape that measures at the 2x rate)."""
            st = tstate[gt]
            hbs = st["hbs"]
            x_sb = st["x"]
            a = st["a"]
            dg = NGRP - 1
            for g in range(dg):
                nc.vector.tensor_tensor(
                    out=a[:, g, :, :],
                    in0=hbs[:, g, None, :].to_broadcast([128, NXT, TSZ]),
                    in1=x_sb[:, :, :],
                    op=mybir.AluOpType.mult)
            nv = NXT - GP_CX
            nc.vector.tensor_tensor(
                out=a[:, dg, :nv, :],
                in0=hbs[:, dg, None, :].to_broadcast([128, nv, TSZ]),
                in1=x_sb[:, :nv, :],
                op=mybir.AluOpType.mult)

        def do_einsum(gt, c0, c1):
            st = tstate[gt]
            u_ps = st.get("u_ps")
            if u_ps is None:
                u_ps = psum_u.tile([LO, TSZ], FP32, tag="u", name=f"u_{gt}")
                st["u_ps"] = u_ps
            a = st["a"]
            for c in range(c0, c1):
                g, cx = c // NXT, c % NXT
                nc.tensor.matmul(
                    out=u_ps[:],
                    lhsT=wg_sb[:, c * LO:(c + 1) * LO],
                    rhs=a[:, g, cx, :],
                    start=(c == 0), stop=(c == N_CHUNK - 1),
                    skip_group_check=True)

        def do_u_evac(gt):
            st = tstate[gt]
            u_sb = stream3.tile([LO, TSZ], BF16_DT, tag="usb", name=f"usb_{gt}")
            nc.scalar.copy(out=u_sb[:], in_=st["u_ps"][:])
            st["u_sb"] = u_sb

        def do_transposes(gt):
            """Transpose u to edge-major while l-expanding to the 9-segment
            m-layout in one go: the matmul rhs is a 0/1 replication matrix
            rm[(l,o),(m,c)] = (l==l(m))*(o==c). The transposes were already
            weight-load-bound, so the wider F is nearly free on the PE, and
            it lets msgs collapse to a single unit-stride 2x DVE op."""
            st = tstate[gt]
            ut_ps = psum_ut.tile([SUB, 4, F_OUT], BF16_DT, tag="utp",
                                 name=f"utp_{gt}")
            for s in range(4):
                nc.tensor.transpose(
                    out=ut_ps[:, s, :],
                    in_=st["u_sb"][:, s * SUB:(s + 1) * SUB],
                    identity=rm_sb[:])
            ut_sb = stream3.tile([SUB, 4, F_OUT], BF16_DT, tag="utsb",
                                 name=f"utsb_{gt}")
            nc.scalar.copy(ut_sb[:], ut_ps[:])
            st["ut"] = ut_sb

        def do_msgs(gt):
            """msgs = ut_m * atx: one 2x-mode DVE op (both operands m-major
            [p, s, 288], unit stride)."""
            st = tstate[gt]
            msgs_sb = stream3.tile([SUB, 4, F_OUT], BF16_DT, tag="msgs",
                                   name=f"msgs_{gt}")
            nc.vector.tensor_tensor(
                out=msgs_sb[:],
                in0=st["ut"][:],
                in1=st["atx"][:].rearrange("p s m c -> p s (m c)"),
                op=mybir.AluOpType.mult)
            st["msgs"] = msgs_sb

        def do_scatter(gt):
            st = tstate[gt]
            for s in range(4):
                stg = 4 * gt + s
                w = stg // SUBS_PER_WIN
                l = stg % SUBS_PER_WIN
                ws = wstate[w]
                if ws["acc"] is None:
                    ws["acc"] = psum_acc.tile([WIN, F_OUT], FP32, tag="acc",
                                              name=f"acc_w{w}")
                nc.tensor.matmul(out=ws["acc"][:], lhsT=ws["mk"][:, l, :],
                                 rhs=st["msgs"][:, s, :],
                                 start=(l == 0),
                                 stop=(l == SUBS_PER_WIN - 1),
                                 skip_group_check=True)
                if l == SUBS_PER_WIN - 1:
                    osb = stream2.tile([WIN, F_OUT], FP32, tag="osb",
                                       name=f"osb_w{w}")
                    nc.scalar.copy(osb[:], ws["acc"][:])
                    nc.sync.dma_start(out[w * WIN:(w + 1) * WIN, :], osb[:])
                    wstate.pop(w)

        # ---- prologue ----
        load_masks(0)
        load_masks(1)
        for gt in range(5):
            prefetch(gt)
        do_mlp12(0)
        do_mlp12(1)
        do_mlp12(2)
        do_zb(0, 0, NGRP)
        do_zb(1, 0, NGRP)
        do_had_gp(0)
        do_had_gp(1)
        do_had_dve(0)

        # ---- main loop ----
        for gt in range(N_TILES):
            if gt % 4 == 0 and gt // 4 + 2 < WINS_PER_CORE:
                load_masks(gt // 4 + 2)
            if gt + 5 < N_TILES:
                prefetch(gt + 5)

            # PE order is tuned so every cross-engine producer lands ahead
            # of its consumer: zb feeds Act evacs feeding the DVE Hadamard;
            # the einsum fills PE time while Act/DVE catch up.
            if gt + 2 < N_TILES:
                do_zb(gt + 2, 0, NGRP - 1)
            if gt + 1 < N_TILES:
                do_had_dve(gt + 1)
            if gt + 3 < N_TILES:
                do_mlp1(gt + 3)
            if gt + 2 < N_TILES:
                do_zb(gt + 2, NGRP - 1, NGRP)
                do_had_gp(gt + 2)
            do_einsum(gt, 0, 4)
            if gt >= 1:
                do_transposes(gt - 1)
            do_einsum(gt, 4, 8)
            if gt >= 2:
                do_scatter(gt - 2)
            if gt >= 1:
                do_msgs(gt - 1)
            do_einsum(gt, 8, 12)
            if gt + 3 < N_TILES:
                do_mlp2(gt + 3)
            do_einsum(gt, 12, N_CHUNK)
            do_u_evac(gt)
            if gt >= 2:
                tstate.pop(gt - 2)

        # ---- drain ----
        do_transposes(N_TILES - 1)
        do_msgs(N_TILES - 1)
        do_scatter(N_TILES - 2)
        do_scatter(N_TILES - 1)

    nc.compile()
    return nc


def _pack_windows(receivers):
    """LPT bin-packing of nodes into N_WINS windows of <=WIN nodes and
    <=WIN_E edges. Returns (win_of_node, slot_of_node)."""
    counts = np.bincount(receivers, minlength=N_NODES)
    order = np.argsort(-counts, kind="stable")
    win_of_node = np.empty(N_NODES, np.int64)
    slot_of_node = np.empty(N_NODES, np.int64)
    heap = [(0, w) for w in range(N_WINS)]
    heapq.heapify(heap)
    nnodes = np.zeros(N_WINS, np.int64)
    loads = np.zeros(N_WINS, np.int64)
    for n in order:
        while True:
            load, w = heapq.heappop(heap)
            if nnodes[w] < WIN:
                break
        win_of_node[n] = w
        slot_of_node[n] = nnodes[w]
        nnodes[w] += 1
        loads[w] = load + counts[n]
        if nnodes[w] < WIN:
            heapq.heappush(heap, (loads[w], w))
    assert loads.max() <= WIN_E, f"window overflow: {loads.max()} > {WIN_E}"
    return win_of_node, slot_of_node


def _host_prep(node_feats, edge_attrs, edge_feats, senders, receivers,
               W1, W2, W3, Wgen):
    senders = np.asarray(senders).astype(np.int64)
    receivers = np.asarray(receivers).astype(np.int64)
    node_feats = np.asarray(node_feats, dtype=np.float32)
    edge_attrs = np.asarray(edge_attrs, dtype=np.float32)
    edge_feats = np.asarray(edge_feats, dtype=np.float32)

    win_of_node, slot_of_node = _pack_windows(receivers)

    e_win = win_of_node[receivers]
    order = np.argsort(e_win, kind="stable")
    counts = np.bincount(e_win, minlength=N_WINS)
    starts = np.zeros(N_WINS + 1, np.int64)
    np.cumsum(counts, out=starts[1:])

    E_TOT = N_CORES * E_CORE
    ef_s = np.zeros((E_TOT, RADIAL), np.float32)
    at_s = np.zeros((E_TOT, NSH), np.float32)
    rl_s = np.full(E_TOT, -1, np.int64)          # -1 -> all-zero mask row
    sd_s = np.zeros(E_TOT, np.int64)

    slot_base = np.arange(N_WINS) * WIN_E
    within = np.arange(len(order)) - starts[e_win[order]]
    slots = slot_base[e_win[order]] + within
    ef_s[slots] = edge_feats[order]
    at_s[slots] = edge_attrs[order] * np.float32(1.0 / np.sqrt(AVG_NUM_NEIGHBORS))
    rl_s[slots] = slot_of_node[receivers[order]]
    sd_s[slots] = senders[order]

    nf_b = node_feats.astype(BF16)

    w1 = (W1 * (1.0 / np.sqrt(RADIAL))).astype(BF16)
    w2 = (W2 * (1.0 / np.sqrt(HID))).astype(BF16)
    w3 = np.asarray(W3, dtype=np.float32) * np.float32(1.0 / np.sqrt(HID))
    p = np.arange(128)
    # w3r[j, g*128+p] = W3[j, KA*g + p//IB]
    w3r = np.zeros((HID, NGRP, 128), np.float32)
    for g in range(NGRP):
        w3r[:, g, :] = w3[:, KA * g + p // IB]
    w3r = w3r.reshape(HID, NGRP * 128).astype(BF16)

    wgen = np.asarray(Wgen, dtype=np.float32) * np.float32(1.0 / np.sqrt(HID * C))
    wg = np.zeros((N_CHUNK, 128, NL, C), np.float32)
    for g in range(NGRP):
        for cx in range(NXT):
            wg[g * NXT + cx] = wgen[KA * g + p // IB][
                p, :, :, IB * cx + p % IB].reshape(128, NL, C)
    wg = wg.reshape(N_CHUNK, 128, LO).transpose(1, 0, 2).reshape(128, N_CHUNK * LO)
    wg = wg.astype(BF16)

    # rm[(l,o), (m,c)] = (l == l(m)) * (o == c): transpose-with-l-expansion
    lmap = np.repeat(np.arange(NL), L_DIMS)            # [9] -> l of each m
    rm = np.zeros((NL, C, NSH, C), np.float32)
    for m in range(NSH):
        rm[lmap[m], :, m, :] = np.eye(C, dtype=np.float32)
    rm = rm.reshape(LO, F_OUT).astype(BF16)

    # masks: mk[p, (w_local, st, n)] per core
    rl_view = rl_s.reshape(N_WINS, SUBS_PER_WIN, SUB)     # [w, st, p]
    n_idx = np.arange(WIN, dtype=np.int64)
    mk_all = (rl_view[:, :, :, None] == n_idx).astype(BF16)  # [w, st, p, n]
    mk_all = mk_all.transpose(0, 2, 1, 3)                  # [w, p, st, n]

    in_maps = []
    for m in range(N_CORES):
        sl = slice(m * E_CORE, (m + 1) * E_CORE)
        ef_c = ef_s[sl]
        at_c = at_s[sl]
        sd_c = sd_s[sl]
        x_c = nf_b[sd_c]                                   # [E_CORE, 32] bf16
        # xs[p, cx, e] = x[e, IB*cx + p%IB], tile-block-major
        xg = x_c.T.reshape(NXT, IB, E_CORE)
        xs_all = np.tile(xg, (1, 128 // IB, 1))            # [cx, 128, e]
        xs_all = xs_all.transpose(1, 0, 2)                 # [128, cx, e]
        blocks = []
        for gt in range(N_TILES):
            b0 = gt * TSZ
            blocks.append(xs_all[:, :, b0:b0 + TSZ].reshape(128, -1))
        xs_c = np.ascontiguousarray(np.concatenate(blocks, axis=1))
        # atx[p, (st, m, c)] = at[st*128+p, m] replicated over c
        at_v = at_c.reshape(N_SUBTILES, SUB, NSH).transpose(1, 0, 2)  # [p, st, m]
        atx_c = np.broadcast_to(at_v[:, :, :, None],
                                (SUB, N_SUBTILES, NSH, C))
        atx_c = np.ascontiguousarray(atx_c.reshape(SUB, -1)).astype(BF16)
        mk_c = np.ascontiguousarray(
            mk_all[m * WINS_PER_CORE:(m + 1) * WINS_PER_CORE]
            .transpose(1, 0, 2, 3).reshape(SUB, -1))
        in_maps.append({
            "ef": np.ascontiguousarray(ef_c.T).astype(BF16),
            "xs": xs_c,
            "atx": atx_c,
            "mk": mk_c,
            "w1": w1, "w2": w2, "w3r": w3r, "wg": wg, "rm": rm,
        })
    return in_maps, win_of_node, slot_of_node


def kernel(node_feats, edge_attrs, edge_feats, senders, receivers,
           W1, W2, W3, Wgen):
    in_maps, win_of_node, slot_of_node = _host_prep(
        node_feats, edge_attrs, edge_feats, senders, receivers,
        W1, W2, W3, Wgen)
    if "nc" not in _CACHED:
        _CACHED["nc"] = _build_nc()
    nc = _CACHED["nc"]
    res = run_bass_kernel_spmd(nc, in_maps, core_ids=list(range(N_CORES)))
    outs = [res.results[m]["out"] for m in range(N_CORES)]
    full = np.concatenate(outs, axis=0)                    # [10240, 288]
    rows = win_of_node * WIN + slot_of_node                # node -> row
    res_rows = full[rows]                                  # [10000, 288]
    out = res_rows.reshape(N_NODES, NSH, C).transpose(0, 2, 1)
    return np.ascontiguousarray(out.astype(np.float32))


# revision 29
# speedup vs baseline: 1.0573x; 1.0573x over previous
"""MessagePassingConvolution kernel for 8 Trainium2 NeuronCores (v2).

Strategy (all-bf16, deep software pipeline, engine-balanced):
  - Host: nodes are LPT bin-packed into 80 balanced windows of <=128 nodes
    and <=2048 edges (16 subtiles of 128; data max ~2010), so the per-core
    edge budget drops to 20480 (vs 21760 with naive contiguous windows)
    and every window is exactly 4 tiles of 512 edges - no ragged tiles,
    no window-straddling tiles. Core m owns windows 10m..10m+9; the host
    inverse-permutes the output rows at the end.
  - The per-edge einsum u[lo,e] = sum_ki Wg[ki,lo] h3[k,e] x[i,e] uses the
    ki -> (group, partition) split k = 16g + p//8, i = 8cx + p%8 (IB=8):
    4 x-replication tables streamed from host (xs, 2 bytes * 512 rows/edge
    = half the DMA of the IB=4 split) and 4 h-replication patterns that
    fold into the third MLP layer: zb_g = (W3 R_g).T @ h2 on the PE, then
    one Act silu per group evacuates psum->SBUF bf16 (silu commutes with
    the 0/1 replication R). The DVE Hadamard A = hbs * xs runs in 2x
    16-bit all-SBUF mode as TWO ops per tile (groups 0-2 merged into one
    [128,3,4,512] op); group 3's last 2 chunks run on the otherwise-idle
    GpSimd engine. The PE then accumulates u[96,512] += Wg_c.T @ A_c over
    the 16 chunks.
  - Output side: GpSimd evacuates u to bf16, PE transposes to edge-major,
    DVE multiplies by host-pre-expanded edge_attrs (atx, c-replicated so
    the multiply runs in 2x mode with unit-stride operands), and the
    scatter accumulates psum_acc[128,288] += S_st.T @ msgs_st over a
    window's 16 subtiles. Scatter masks are host-precomputed and DMA'd
    (beats on-chip is_equal, which runs at 1x due to a stride-0 operand).
  - Every pipeline stage has a full iteration of slack: iteration gt runs
    mlp(gt+3), zb+silu(gt+2), Hadamard(gt+1), einsum(gt), transpose/msgs
    (gt-1), scatter(gt-2). Bulk xs DMA rides the GpSimd queue (25ns
    dispatch vs 565ns on sync); ef/atx/masks/out ride sync/scalar.
  - Output: per-core [1280, 288] slices -> concat -> inverse node
    permutation -> [10000, 32, 9].
"""

import sys
import numpy as np
import heapq
from contextlib import ExitStack

sys.path.insert(0, "/opt/trn_rl_repo")

import concourse.bass as bass  # noqa: E402
import concourse.bacc as bacc  # noqa: E402
import concourse.mybir as mybir  # noqa: E402
import concourse.tile as tile  # noqa: E402
from concourse.bass_utils import run_bass_kernel_spmd  # noqa: E402

import ml_dtypes  # noqa: E402

BF16 = ml_dtypes.bfloat16

# ---- problem constants (hardcoded per spec) ----
N_NODES = 10000
N_EDGES = 160000
C = 32
RADIAL = 8
HID = 64
NL = 3
L_DIMS = (1, 3, 5)
NSH = 9
AVG_NUM_NEIGHBORS = 16.0

N_CORES = 8
WIN = 128                      # nodes per window (psum partitions)
WINS_PER_CORE = 10
N_WINS = N_CORES * WINS_PER_CORE            # 80
SUB = 128                      # edges per subtile
SUBS_PER_WIN = 16              # window edge budget = 2048 (balanced packing)
WIN_E = SUB * SUBS_PER_WIN     # 2048
E_CORE = WIN_E * WINS_PER_CORE              # 20480
N_SUBTILES = WINS_PER_CORE * SUBS_PER_WIN   # 160
TSZ = 512                      # edges per tile (4 subtiles)
N_TILES = N_SUBTILES // 4                   # 40 (window = exactly 4 tiles)

IB = 4                         # i-values per chunk partition period
KA = 128 // IB                 # k-rows per group = 32
NXT = C // IB                  # 8 x-replication tables
NGRP = HID // KA               # 2 h-groups
N_CHUNK = NGRP * NXT           # 16 ki-chunks
GP_CX = 0                      # chunks of the last group computed on GpSimd
                               # (0: measured GpSimd offload inflates the
                               # concurrent DVE op by more than it saves)
LO = NL * C                    # 96
F_OUT = NSH * C                # 288

FP32 = mybir.dt.float32
BF16_DT = mybir.dt.bfloat16

_CACHED = {}

ACT_FUNC = mybir.ActivationFunctionType.Silu


def _build_nc():
    nc = bacc.Bacc()

    ef = nc.dram_tensor("ef", [RADIAL, E_CORE], BF16_DT, kind="ExternalInput")
    xs = nc.dram_tensor("xs", [128, NXT * E_CORE], BF16_DT, kind="ExternalInput")
    atx = nc.dram_tensor("atx", [SUB, N_SUBTILES * NSH * C], BF16_DT,
                         kind="ExternalInput")
    mk = nc.dram_tensor("mk", [SUB, WINS_PER_CORE * SUBS_PER_WIN * WIN], BF16_DT,
                        kind="ExternalInput")
    w1 = nc.dram_tensor("w1", [RADIAL, HID], BF16_DT, kind="ExternalInput")
    w2 = nc.dram_tensor("w2", [HID, HID], BF16_DT, kind="ExternalInput")
    w3r = nc.dram_tensor("w3r", [HID, NGRP * 128], BF16_DT, kind="ExternalInput")
    wg = nc.dram_tensor("wg", [128, N_CHUNK * LO], BF16_DT, kind="ExternalInput")
    ident = nc.dram_tensor("ident", [128, 128], BF16_DT, kind="ExternalInput")
    out = nc.dram_tensor("out", [WIN * WINS_PER_CORE, F_OUT], FP32,
                         kind="ExternalOutput")

    with tile.TileContext(nc) as tc, ExitStack() as ctx:
        const_p = ctx.enter_context(tc.tile_pool(name="const", bufs=1))
        stream5 = ctx.enter_context(tc.tile_pool(name="s5", bufs=5))
        stream3 = ctx.enter_context(tc.tile_pool(name="s3", bufs=3))
        stream2 = ctx.enter_context(tc.tile_pool(name="s2", bufs=3))
        win_p = ctx.enter_context(tc.tile_pool(name="win", bufs=4))
        psum_mlp = ctx.enter_context(tc.tile_pool(name="pmlp", bufs=1, space="PSUM"))
        psum_zb = ctx.enter_context(tc.tile_pool(name="pzb", bufs=3, space="PSUM"))
        psum_u = ctx.enter_context(tc.tile_pool(name="pu", bufs=2, space="PSUM"))
        psum_ut = ctx.enter_context(tc.tile_pool(name="put", bufs=1, space="PSUM"))
        psum_acc = ctx.enter_context(tc.tile_pool(name="pacc", bufs=1, space="PSUM"))

        # ---- one-time constants ----
        w1_sb = const_p.tile([RADIAL, HID], BF16_DT)
        nc.sync.dma_start(w1_sb[:], w1[:])
        w2_sb = const_p.tile([HID, HID], BF16_DT)
        nc.sync.dma_start(w2_sb[:], w2[:])
        ident_sb = const_p.tile([128, 128], BF16_DT)
        nc.sync.dma_start(ident_sb[:], ident[:])
        w3r_sb = const_p.tile([HID, NGRP * 128], BF16_DT)
        nc.scalar.dma_start(w3r_sb[:], w3r[:])
        wg_sb = const_p.tile([128, N_CHUNK * LO], BF16_DT)
        nc.scalar.dma_start(wg_sb[:], wg[:])

        tstate = {}   # gt -> dict(ef, x, atx, h1, h2, hbs, a, u_sb, ut, msgs)
        wstate = {}   # w -> dict(mk, acc)
        lofs = (0, 1, 4)

        def prefetch(gt):
            """ef/atx on sync queue, bulk xs on the cheap gpsimd queue."""
            base = gt * TSZ
            ef_sb = stream5.tile([RADIAL, TSZ], BF16_DT, tag="ef", name=f"ef_{gt}")
            nc.sync.dma_start(ef_sb[:], ef[:, base:base + TSZ])
            x_sb = stream5.tile([128, NXT, TSZ], BF16_DT, tag="x", name=f"x_{gt}")
            nc.gpsimd.dma_start(
                x_sb[:].rearrange("p c e -> p (c e)"),
                xs[:, NXT * base:NXT * (base + TSZ)])
            atx_sb = stream5.tile([SUB, 4, NSH, C], BF16_DT, tag="atx",
                                  name=f"atx_{gt}")
            nc.sync.dma_start(
                atx_sb[:].rearrange("p s m c -> p (s m c)"),
                atx[:, gt * 4 * NSH * C:(gt + 1) * 4 * NSH * C])
            tstate[gt] = dict(ef=ef_sb, x=x_sb, atx=atx_sb)

        def load_masks(w):
            mk_sb = win_p.tile([SUB, SUBS_PER_WIN, WIN], BF16_DT, tag="mk",
                               name=f"mk_w{w}")
            nc.scalar.dma_start(
                mk_sb[:].rearrange("p s n -> p (s n)"),
                mk[:, w * SUBS_PER_WIN * WIN:(w + 1) * SUBS_PER_WIN * WIN])
            wstate[w] = dict(mk=mk_sb, acc=None)

        def do_mlp1(gt):
            """Layer 1 of the radial MLP for tile gt (PE + Act silu)."""
            st = tstate[gt]
            z1 = psum_mlp.tile([HID, TSZ], FP32, tag="z", name=f"z1_{gt}")
            nc.tensor.matmul(out=z1[:], lhsT=w1_sb[:], rhs=st["ef"][:],
                             start=True, stop=True, skip_group_check=True)
            h1 = stream3.tile([HID, TSZ], BF16_DT, tag="h1", name=f"h1_{gt}")
            nc.scalar.activation(h1[:], z1[:], ACT_FUNC)
            st["h1"] = h1

        def do_mlp2(gt):
            st = tstate[gt]
            z2 = psum_mlp.tile([HID, TSZ], FP32, tag="z", name=f"z2_{gt}")
            nc.tensor.matmul(out=z2[:], lhsT=w2_sb[:], rhs=st["h1"][:],
                             start=True, stop=True, skip_group_check=True)
            h2 = stream3.tile([HID, TSZ], BF16_DT, tag="h2", name=f"h2_{gt}")
            nc.scalar.activation(h2[:], z2[:], ACT_FUNC)
            st["h2"] = h2

        def do_mlp12(gt):
            do_mlp1(gt)
            do_mlp2(gt)

        def do_zb(gt, g0, g1):
            """MLP layer 3 fused with h-replication: zb_g = (W3 R_g).T @ h2,
            Act silu evacuates each group to SBUF bf16 (silu commutes with
            the 0/1 replication)."""
            st = tstate[gt]
            if g0 == 0:
                st["hbs"] = stream3.tile([128, NGRP, TSZ], BF16_DT, tag="hbs",
                                         name=f"hbs_{gt}")
            hbs = st["hbs"]
            for g in range(g0, g1):
                zb = psum_zb.tile([128, TSZ], FP32, tag="zb", name=f"zb_{gt}_{g}")
                nc.tensor.matmul(out=zb[:],
                                 lhsT=w3r_sb[:, g * 128:(g + 1) * 128],
                                 rhs=st["h2"][:],
                                 start=True, stop=True, skip_group_check=True)
                nc.scalar.activation(hbs[:, g, :], zb[:], ACT_FUNC)

        def do_had_gp(gt):
            """GpSimd's share of the Hadamard (last chunk of the last
            group), staggered one iteration ahead of the DVE share so the
            two engines never contend for the same SBUF tiles."""
            st = tstate[gt]
            a = stream2.tile([128, NGRP, NXT, TSZ], BF16_DT, tag="a",
                             name=f"a_{gt}")
            if GP_CX:
                dg = NGRP - 1
                nv = NXT - GP_CX
                nc.gpsimd.tensor_tensor(
                    out=a[:, dg, nv:, :],
                    in0=st["hbs"][:, dg, None, :].to_broadcast(
                        [128, GP_CX, TSZ]),
                    in1=st["x"][:, nv:, :],
                    op=mybir.AluOpType.mult)
            st["a"] = a

        def do_had_dve(gt):
            """DVE share of the Hadamard: one 3D 2x-mode op per group (the
            exact AP shape that measures at the 2x rate)."""
            st = tstate[gt]
            hbs = st["hbs"]
            x_sb = st["x"]
            a = st["a"]
            dg = NGRP - 1
            for g in range(dg):
                nc.vector.tensor_tensor(
                    out=a[:, g, :, :],
                    in0=hbs[:, g, None, :].to_broadcast([128, NXT, TSZ]),
                    in1=x_sb[:, :, :],
                    op=mybir.AluOpType.mult)
            nv = NXT - GP_CX
            nc.vector.tensor_tensor(
                out=a[:, dg, :nv, :],
                in0=hbs[:, dg, None, :].to_broadcast([128, nv, TSZ]),
                in1=x_sb[:, :nv, :],
                op=mybir.AluOpType.mult)

        def do_einsum(gt, c0, c1):
            st = tstate[gt]
            u_ps = st.get("u_ps")
            if u_ps is None:
                u_ps = psum_u.tile([LO, TSZ], FP32, tag="u", name=f"u_{gt}")
                st["u_ps"] = u_ps
            a = st["a"]
            for c in range(c0, c1):
                g, cx = c // NXT, c % NXT
                nc.tensor.matmul(
                    out=u_ps[:],
                    lhsT=wg_sb[:, c * LO:(c + 1) * LO],
                    rhs=a[:, g, cx, :],
                    start=(c == 0), stop=(c == N_CHUNK - 1),
                    skip_group_check=True)

        def do_u_evac(gt):
            st = tstate[gt]
            u_sb = stream3.tile([LO, TSZ], BF16_DT, tag="usb", name=f"usb_{gt}")
            nc.scalar.copy(out=u_sb[:], in_=st["u_ps"][:])
            st["u_sb"] = u_sb

        def do_transposes(gt):
            st = tstate[gt]
            ut_ps = psum_ut.tile([SUB, 4, LO], BF16_DT, tag="utp",
                                 name=f"utp_{gt}")
            for s in range(4):
                nc.tensor.transpose(
                    out=ut_ps[:, s, :],
                    in_=st["u_sb"][:, s * SUB:(s + 1) * SUB],
                    identity=ident_sb[:LO, :LO])
            ut_sb = stream3.tile([SUB, 4, LO], BF16_DT, tag="utsb",
                                 name=f"utsb_{gt}")
            nc.scalar.copy(ut_sb[:], ut_ps[:])
            st["ut"] = ut_sb

        def do_msgs(gt):
            """msgs = uT * atx, l=1,2 segments in 2x-mode DVE ops (atx is
            c-replicated on host so every operand is unit-stride); the small
            l=0 segment rides GpSimd."""
            st = tstate[gt]
            ut_sb = st["ut"]
            atx_sb = st["atx"]
            msgs_sb = stream3.tile([SUB, 4, F_OUT], BF16_DT, tag="msgs", bufs=4,
                                   name=f"msgs_{gt}")
            for l in range(NL):
                dim = L_DIMS[l]
                nc.vector.tensor_tensor(
                    out=msgs_sb[:, :, lofs[l] * C:(lofs[l] + dim) * C].rearrange(
                        "p s (m c) -> p s m c", c=C),
                    in0=ut_sb[:, :, None, l * C:(l + 1) * C].to_broadcast(
                        [SUB, 4, dim, C]),
                    in1=atx_sb[:, :, lofs[l]:lofs[l] + dim, :],
                    op=mybir.AluOpType.mult)
            st["msgs"] = msgs_sb

        def do_scatter(gt):
            st = tstate[gt]
            for s in range(4):
                stg = 4 * gt + s
                w = stg // SUBS_PER_WIN
                l = stg % SUBS_PER_WIN
                ws = wstate[w]
                if ws["acc"] is None:
                    ws["acc"] = psum_acc.tile([WIN, F_OUT], FP32, tag="acc",
                                              name=f"acc_w{w}")
                nc.tensor.matmul(out=ws["acc"][:], lhsT=ws["mk"][:, l, :],
                                 rhs=st["msgs"][:, s, :],
                                 start=(l == 0),
                                 stop=(l == SUBS_PER_WIN - 1),
                                 skip_group_check=True)
                if l == SUBS_PER_WIN - 1:
                    osb = stream2.tile([WIN, F_OUT], FP32, tag="osb",
                                       name=f"osb_w{w}")
                    nc.scalar.copy(osb[:], ws["acc"][:])
                    nc.sync.dma_start(out[w * WIN:(w + 1) * WIN, :], osb[:])
                    wstate.pop(w)

        # ---- prologue ----
        load_masks(0)
        load_masks(1)
        for gt in range(5):
            prefetch(gt)
        do_mlp12(0)
        do_mlp12(1)
        do_mlp12(2)
        do_zb(0, 0, NGRP)
        do_zb(1, 0, NGRP)
        do_had_gp(0)
        do_had_gp(1)
        do_had_dve(0)

        # ---- main loop ----
        for gt in range(N_TILES):
            if gt % 4 == 0 and gt // 4 + 2 < WINS_PER_CORE:
                load_masks(gt // 4 + 2)
            if gt + 5 < N_TILES:
                prefetch(gt + 5)

            # PE order is tuned so every cross-engine producer lands ahead
            # of its consumer: zb feeds Act evacs feeding the DVE Hadamard;
            # the einsum fills PE time while Act/DVE catch up.
            if gt + 2 < N_TILES:
                do_zb(gt + 2, 0, NGRP - 1)
            if gt + 1 < N_TILES:
                do_had_dve(gt + 1)
            if gt + 3 < N_TILES:
                do_mlp1(gt + 3)
            if gt + 2 < N_TILES:
                do_zb(gt + 2, NGRP - 1, NGRP)
                do_had_gp(gt + 2)
            do_einsum(gt, 0, 4)
            if gt >= 1:
                do_transposes(gt - 1)
            do_einsum(gt, 4, 8)
            # scatter lags 3 tiles (not 2): its msgs input is then a full
            # iteration old, so the first scatter matmul never waits on the
            # DVE semaphore - a ~65ns near-miss there resets the PE p-state
            # (3us at 1.2GHz), costing ~30us over the run.
            if gt >= 3:
                do_scatter(gt - 3)
            if gt >= 1:
                do_msgs(gt - 1)
            do_einsum(gt, 8, 12)
            if gt + 3 < N_TILES:
                do_mlp2(gt + 3)
            do_einsum(gt, 12, N_CHUNK)
            do_u_evac(gt)
            if gt >= 3:
                tstate.pop(gt - 3)

        # ---- drain ----
        do_transposes(N_TILES - 1)
        do_msgs(N_TILES - 1)
        do_scatter(N_TILES - 3)
        do_scatter(N_TILES - 2)
        do_scatter(N_TILES - 1)

    nc.compile()
    return nc


def _pack_windows(receivers):
    """LPT bin-packing of nodes into N_WINS windows of <=WIN nodes and
    <=WIN_E edges. Returns (win_of_node, slot_of_node)."""
    counts = np.bincount(receivers, minlength=N_NODES)
    order = np.argsort(-counts, kind="stable")
    win_of_node = np.empty(N_NODES, np.int64)
    slot_of_node = np.empty(N_NODES, np.int64)
    heap = [(0, w) for w in range(N_WINS)]
    heapq.heapify(heap)
    nnodes = np.zeros(N_WINS, np.int64)
    loads = np.zeros(N_WINS, np.int64)
    for n in order:
        while True:
            load, w = heapq.heappop(heap)
            if nnodes[w] < WIN:
                break
        win_of_node[n] = w
        slot_of_node[n] = nnodes[w]
        nnodes[w] += 1
        loads[w] = load + counts[n]
        if nnodes[w] < WIN:
            heapq.heappush(heap, (loads[w], w))
    assert loads.max() <= WIN_E, f"window overflow: {loads.max()} > {WIN_E}"
    return win_of_node, slot_of_node


def _host_prep(node_feats, edge_attrs, edge_feats, senders, receivers,
               W1, W2, W3, Wgen):
    senders = np.asarray(senders).astype(np.int64)
    receivers = np.asarray(receivers).astype(np.int64)
    node_feats = np.asarray(node_feats, dtype=np.float32)
    edge_attrs = np.asarray(edge_attrs, dtype=np.float32)
    edge_feats = np.asarray(edge_feats, dtype=np.float32)

    win_of_node, slot_of_node = _pack_windows(receivers)

    e_win = win_of_node[receivers]
    order = np.argsort(e_win, kind="stable")
    counts = np.bincount(e_win, minlength=N_WINS)
    starts = np.zeros(N_WINS + 1, np.int64)
    np.cumsum(counts, out=starts[1:])

    E_TOT = N_CORES * E_CORE
    ef_s = np.zeros((E_TOT, RADIAL), np.float32)
    at_s = np.zeros((E_TOT, NSH), np.float32)
    rl_s = np.full(E_TOT, -1, np.int64)          # -1 -> all-zero mask row
    sd_s = np.zeros(E_TOT, np.int64)

    slot_base = np.arange(N_WINS) * WIN_E
    within = np.arange(len(order)) - starts[e_win[order]]
    slots = slot_base[e_win[order]] + within
    ef_s[slots] = edge_feats[order]
    at_s[slots] = edge_attrs[order] * np.float32(1.0 / np.sqrt(AVG_NUM_NEIGHBORS))
    rl_s[slots] = slot_of_node[receivers[order]]
    sd_s[slots] = senders[order]

    nf_b = node_feats.astype(BF16)

    w1 = (W1 * (1.0 / np.sqrt(RADIAL))).astype(BF16)
    w2 = (W2 * (1.0 / np.sqrt(HID))).astype(BF16)
    w3 = np.asarray(W3, dtype=np.float32) * np.float32(1.0 / np.sqrt(HID))
    p = np.arange(128)
    # w3r[j, g*128+p] = W3[j, KA*g + p//IB]
    w3r = np.zeros((HID, NGRP, 128), np.float32)
    for g in range(NGRP):
        w3r[:, g, :] = w3[:, KA * g + p // IB]
    w3r = w3r.reshape(HID, NGRP * 128).astype(BF16)

    wgen = np.asarray(Wgen, dtype=np.float32) * np.float32(1.0 / np.sqrt(HID * C))
    wg = np.zeros((N_CHUNK, 128, NL, C), np.float32)
    for g in range(NGRP):
        for cx in range(NXT):
            wg[g * NXT + cx] = wgen[KA * g + p // IB][
                p, :, :, IB * cx + p % IB].reshape(128, NL, C)
    wg = wg.reshape(N_CHUNK, 128, LO).transpose(1, 0, 2).reshape(128, N_CHUNK * LO)
    wg = wg.astype(BF16)

    ident = np.eye(128, dtype=np.float32).astype(BF16)

    # masks: mk[p, (w_local, st, n)] per core
    rl_view = rl_s.reshape(N_WINS, SUBS_PER_WIN, SUB)     # [w, st, p]
    n_idx = np.arange(WIN, dtype=np.int64)
    mk_all = (rl_view[:, :, :, None] == n_idx).astype(BF16)  # [w, st, p, n]
    mk_all = mk_all.transpose(0, 2, 1, 3)                  # [w, p, st, n]

    in_maps = []
    for m in range(N_CORES):
        sl = slice(m * E_CORE, (m + 1) * E_CORE)
        ef_c = ef_s[sl]
        at_c = at_s[sl]
        sd_c = sd_s[sl]
        x_c = nf_b[sd_c]                                   # [E_CORE, 32] bf16
        # xs[p, cx, e] = x[e, IB*cx + p%IB], tile-block-major
        xg = x_c.T.reshape(NXT, IB, E_CORE)
        xs_all = np.tile(xg, (1, 128 // IB, 1))            # [cx, 128, e]
        xs_all = xs_all.transpose(1, 0, 2)                 # [128, cx, e]
        blocks = []
        for gt in range(N_TILES):
            b0 = gt * TSZ
            blocks.append(xs_all[:, :, b0:b0 + TSZ].reshape(128, -1))
        xs_c = np.ascontiguousarray(np.concatenate(blocks, axis=1))
        # atx[p, (st, m, c)] = at[st*128+p, m] replicated over c
        at_v = at_c.reshape(N_SUBTILES, SUB, NSH).transpose(1, 0, 2)  # [p, st, m]
        atx_c = np.broadcast_to(at_v[:, :, :, None],
                                (SUB, N_SUBTILES, NSH, C))
        atx_c = np.ascontiguousarray(atx_c.reshape(SUB, -1)).astype(BF16)
        mk_c = np.ascontiguousarray(
            mk_all[m * WINS_PER_CORE:(m + 1) * WINS_PER_CORE]
            .transpose(1, 0, 2, 3).reshape(SUB, -1))
        in_maps.append({
            "ef": np.ascontiguousarray(ef_c.T).astype(BF16),
            "xs": xs_c,
            "atx": atx_c,
            "mk": mk_c,
            "w1": w1, "w2": w2, "w3r": w3r, "wg": wg, "ident": ident,
        })
    return in_maps, win_of_node, slot_of_node


def kernel(node_feats, edge_attrs, edge_feats, senders, receivers,
           W1, W2, W3, Wgen):
    in_maps, win_of_node, slot_of_node = _host_prep(
        node_feats, edge_attrs, edge_feats, senders, receivers,
        W1, W2, W3, Wgen)
    if "nc" not in _CACHED:
        _CACHED["nc"] = _build_nc()
    nc = _CACHED["nc"]
    res = run_bass_kernel_spmd(nc, in_maps, core_ids=list(range(N_CORES)))
    outs = [res.results[m]["out"] for m in range(N_CORES)]
    full = np.concatenate(outs, axis=0)                    # [10240, 288]
    rows = win_of_node * WIN + slot_of_node                # node -> row
    res_rows = full[rows]                                  # [10000, 288]
    out = res_rows.reshape(N_NODES, NSH, C).transpose(0, 2, 1)
    return np.ascontiguousarray(out.astype(np.float32))
